# revision 2
# baseline (speedup 1.0000x reference)
"""GQA attention block (B=1, S=2048, D=2048, H=32, G=8, HD=64) on 8 trn2 cores.

v4: like v3 (all-projections-first, rsqrt cluster, tanh gate, causal
sliced attention) but attention/rope/gating run on 1024-wide sq slices:
exp instructions are ~2x wider (the 352-cycle ACT fixed cost halves),
single-head ctx passes, and the psum budget is sc 2x(2 banks, shared
with the rope broadcast tiles) + cx 1x(2 banks) + pj 2x(1 bank).
"""

import math
from contextlib import ExitStack
import numpy as np
import ml_dtypes

import concourse.bass as bass
import concourse.tile as tile
from concourse import bacc, mybir
from concourse.bass_utils import run_bass_kernel_spmd
from concourse.masks import make_identity

BF16 = mybir.dt.bfloat16
F32 = mybir.dt.float32
NBF = ml_dtypes.bfloat16

S = 2048
D = 2048
H = 32
G = 8
HD = 64
NCORE = 8
NHL = H // NCORE          # 4 local q heads
EL = NHL * HD             # 256 local ctx features
ET = 640                  # 256 q + 64 k + 64 v + 256 gate
P = 128
ND = D // P               # 16 d-chunks
SQ = 512                  # projection slice width (psum bank)
NSQ = S // SQ             # 4 proj slices
AQ = 1024                 # attention slice width
NAQ = S // AQ             # 2 attention slices
SCALE = HD ** -0.5
EPS = 1e-6
AF = mybir.ActivationFunctionType


class Ctx:
    pass


def _load_persistent(nc, g, aps):
    pp = g.pp
    dmae = [nc.sync, nc.scalar]   # two HWDGE queues

    g.wts = []
    for i in range(ND):
        t = pp.tile([P, ET], BF16, tag=f"wts{i}", name=f"wts{i}")
        dmae[i % 2].dma_start(out=t, in_=aps["wt"][i * P:(i + 1) * P, :])
        g.wts.append(t)
    g.xts = [pp.tile([P, S], BF16, tag=f"xt{i}", name=f"xt{i}")
             for i in range(ND)]
    for i in range(ND):
        dmae[i % 2].dma_start(out=g.xts[i][:, 0:SQ],
                              in_=aps["xt"][i * P:(i + 1) * P, 0:SQ])
    for i in range(ND):
        dmae[i % 2].dma_start(out=g.xts[i][:, SQ:S],
                              in_=aps["xt"][i * P:(i + 1) * P, SQ:S])
    g.cqs, g.sqs, g.ckvs, g.skvs = [], [], [], []
    for Q in range(NAQ):
        for li, (lst, src, nm) in enumerate(
                ((g.cqs, "cosq", "cq"), (g.sqs, "sinq", "sq_"),
                 (g.ckvs, "coskv", "ckv"), (g.skvs, "sinkv", "skv"))):
            t = pp.tile([P, AQ], BF16, tag=f"{nm}{Q}", name=f"{nm}{Q}")
            dmae[li % 2].dma_start(out=t, in_=aps[src][:, Q * AQ:(Q + 1) * AQ])
            lst.append(t)
    g.tri_sb = pp.tile([P, P], BF16, tag="tri", name="tri_sb")
    nc.sync.dma_start(out=g.tri_sb, in_=aps["tri"])
    g.wos = []
    for e in range(2):
        t = pp.tile([P, D], BF16, tag=f"wo{e}", name=f"wo{e}")
        dmae[e % 2].dma_start(out=t, in_=aps["wo"][e * P:(e + 1) * P, :])
        g.wos.append(t)
    g.ident = pp.tile([P, P], BF16, tag="ident", name="ident")
    make_identity(nc, g.ident)
    g.ones2 = pp.tile([P, 2], BF16, tag="ones2", name="ones2")
    nc.vector.memset(g.ones2, 0.0)
    nc.vector.memset(g.ones2[0:64, 0:1], 1.0)
    nc.vector.memset(g.ones2[64:P, 1:2], 1.0)
    g.onesk = pp.tile([P, 1], BF16, tag="onesk", name="onesk")
    nc.vector.memset(g.onesk, 0.0)
    nc.vector.memset(g.onesk[0:64, :], 1.0)
    g.epsb = pp.tile([P, 1], F32, tag="epsb", name="epsb")
    nc.vector.memset(g.epsb, EPS)
    g.sel = pp.tile([66, P], BF16, tag="sel", name="sel")
    nc.sync.dma_start(out=g.sel, in_=aps["sel"])

    # persistent intermediates
    g.Cs = [[pp.tile([P, AQ], BF16, tag=f"C{c}_{Q}", name=f"C{c}_{Q}")
             for Q in range(NAQ)] for c in range(3)]
    g.statsb = [pp.tile([65, SQ], F32, tag=f"sb{q}", name=f"sb{q}")
                for q in range(NSQ)]
    g.R = [pp.tile([65, SQ], BF16, tag=f"R{q}", name=f"Rt{q}")
           for q in range(NSQ)]
    g.qh = [[pp.tile([64, AQ], BF16, tag=f"qh{h}_{Q}", name=f"qh{h}_{Q}")
             for Q in range(NAQ)] for h in range(NHL)]
    g.kvr = [pp.tile([P, SQ], BF16, tag=f"kv{q}", name=f"kv{q}")
             for q in range(NSQ)]
    g.vs = []
    for jk in range(ND):
        t = pp.tile([P, HD + 1], BF16, tag=f"v{jk}", name=f"v{jk}")
        nc.vector.memset(t[:, HD:HD + 1], 2.0)   # den column: 2*sum(p)
        g.vs.append(t)
    g.gu = [[pp.tile([P, AQ], BF16, tag=f"gu{p_}_{Q}", name=f"gu{p_}_{Q}")
             for Q in range(NAQ)] for p_ in range(2)]
    g.ctxg = [[pp.tile([P, AQ], BF16, tag=f"cg{p_}_{Q}", name=f"cg{p_}_{Q}")
               for Q in range(NAQ)] for p_ in range(2)]


def _proj_q(nc, g, q, filler=None):
    """Projections for 512-slice q: qkv chunks + stats, gate chunks."""
    filler = iter(filler or ())
    qc = slice(q * SQ, (q + 1) * SQ)
    Q, hf = q // 2, (q % 2) * SQ
    stats = g.cxp.tile([65, SQ], F32, tag="cx", name="stats")
    nc.vector.memset(stats, 0.0)
    for c in range(3):
        Pp = g.pjp.tile([P, SQ], F32, tag="pj", name="pj")
        for i in range(ND):
            nc.tensor.matmul(
                Pp, g.wts[i][:, c * P:(c + 1) * P], g.xts[i][:, qc],
                start=(i == 0), stop=(i == ND - 1))
        nc.vector.tensor_copy(out=g.Cs[c][Q][:, hf:hf + SQ], in_=Pp)
        SQt = g.sqp.tile([P, SQ], BF16, tag="sqt", name="sqt")
        nc.vector.tensor_mul(SQt, g.Cs[c][Q][:, hf:hf + SQ],
                             g.Cs[c][Q][:, hf:hf + SQ])
        if c < 2:
            nc.tensor.matmul(stats[32 * c:32 * c + 2, :], g.ones2, SQt,
                             start=True, stop=True)
        else:
            nc.tensor.matmul(stats[64:65, :], g.onesk, SQt,
                             start=True, stop=True)
        for _ in range(3):
            grp = next(filler, None)
            if grp is not None:
                grp()
    nc.vector.tensor_copy(out=g.statsb[q], in_=stats)
    # gate: u = 1 + tanh(g/2) = 2*sigmoid(g)
    for p_ in range(2):
        Pp = g.pjp.tile([P, SQ], F32, tag="pj", name="pj")
        for i in range(ND):
            nc.tensor.matmul(
                Pp, g.wts[i][:, (3 + p_) * P:(4 + p_) * P], g.xts[i][:, qc],
                start=(i == 0), stop=(i == ND - 1))
        T = g.gtp.tile([P, SQ], BF16, tag="gt", name="gt")
        nc.scalar.activation(T, Pp, AF.Tanh, scale=0.5)
        nc.vector.tensor_scalar_add(g.gu[p_][Q][:, hf:hf + SQ], T, 1.0)
        for _ in range(3):
            grp = next(filler, None)
            if grp is not None:
                grp()


def _rope_pieces(nc, g, Q):
    """Yield small closures: broadcast, norm, RoPE, v transposes for Q."""
    Ns = {}
    for c in range(3):
        def mk_bcast(c=c, hf=None):
            def f():
                N = Ns.setdefault(c, g.nnp.tile([P, AQ], BF16, tag="nn",
                                                name="nn"))
                for h2 in ((0, 1) if hf is None else (hf,)):
                    q = 2 * Q + h2
                    R = g.R[q]
                    co = h2 * SQ
                    rbc = g.scp.tile([P, SQ], F32, tag="sc", name="rbc")
                    if c < 2:
                        b = 32 * c
                        nc.tensor.matmul(rbc, g.sel[b:b + 2, :],
                                         R[b:b + 2, :], start=True, stop=True)
                        nc.vector.tensor_mul(N[:, co:co + SQ],
                                             g.Cs[c][Q][:, co:co + SQ], rbc)
                    else:
                        nc.tensor.matmul(rbc[0:64, :], g.sel[64:65, 0:64],
                                         R[64:65, :], start=True, stop=True)
                        nc.vector.tensor_mul(N[0:64, co:co + SQ],
                                             g.Cs[2][Q][0:64, co:co + SQ],
                                             rbc[0:64, :])
                        nc.vector.tensor_copy(
                            out=N[64:P, co:co + SQ],
                            in_=g.Cs[2][Q][64:P, co:co + SQ])
            return f
        yield mk_bcast(c, 0)
        yield mk_bcast(c, 1)

        def mk_rope(c=c):
            def f():
                N = Ns[c]
                ct = g.cqs[Q] if c < 2 else g.ckvs[Q]
                st_ = g.sqs[Q] if c < 2 else g.skvs[Q]
                T1 = g.ropep.tile([P, AQ], BF16, tag="t1", name="t1")
                nc.vector.tensor_mul(T1, N, ct)
                T2 = g.ropep.tile([P, AQ], BF16, tag="t2", name="t2")
                for blk in range(2):
                    b0 = 64 * blk
                    nc.vector.tensor_mul(T2[b0:b0 + 32, :],
                                         N[b0 + 32:b0 + 64, :],
                                         st_[b0 + 32:b0 + 64, :])
                    nc.vector.tensor_mul(T2[b0 + 32:b0 + 64, :],
                                         N[b0:b0 + 32, :],
                                         st_[b0:b0 + 32, :])
                if c < 2:
                    nc.vector.tensor_add(g.qh[2 * c][Q], T1[0:64, :],
                                         T2[0:64, :])
                    nc.vector.tensor_add(g.qh[2 * c + 1][Q], T1[64:P, :],
                                         T2[64:P, :])
                else:
                    for h2 in range(2):
                        co = h2 * SQ
                        nc.vector.tensor_add(g.kvr[2 * Q + h2][:, :],
                                             T1[:, co:co + SQ],
                                             T2[:, co:co + SQ])
            return f
        yield mk_rope(c)
    for t in range(8):
        def mk_vt(t=t):
            def f():
                jk = 8 * Q + t
                q, tt = jk // 4, jk % 4
                vt_ = g.scp.tile([P, HD], BF16, tag="sc", name="vt")
                nc.tensor.transpose(vt_, g.kvr[q][64:P, tt * P:(tt + 1) * P],
                                    g.ident[64:P, 64:P])
                nc.vector.tensor_copy(out=g.vs[jk][:, 0:HD], in_=vt_)
            return f
        yield mk_vt(t)


def _attn_Q(nc, g, Q, filler=None):
    """Attention + gating for 1024-slice Q, one head per pass."""
    filler = iter(filler or ())
    nks = 8 * Q + 8
    for h in range(NHL):
        p_, hh = h // 2, h % 2
        ctx = g.cxp.tile([65, AQ], F32, tag="cx", name="cx")
        for jk in range(nks):
            dlt = jk - 8 * Q
            c0 = max(dlt, 0) * P
            jq, jc = jk // 4, (jk % 4) * P
            kT = g.kvr[jq][0:64, jc:jc + P]
            ps_s = g.scp.tile([P, AQ], F32, tag="sc", name="sc")
            segs = [(c0, SQ), (SQ, AQ)] if c0 < SQ else [(c0, AQ)]
            for s0, s1 in segs:
                nc.tensor.matmul(ps_s[:, s0:s1], kT, g.qh[h][Q][:, s0:s1],
                                 start=True, stop=True)
            pr = g.prp.tile([P, AQ], BF16, tag="pr", name="pr")
            nc.scalar.activation(pr[:, c0:], ps_s[:, c0:], AF.Exp,
                                 scale=SCALE)
            if dlt >= 0:
                nc.vector.tensor_mul(pr[:, c0:c0 + P], pr[:, c0:c0 + P],
                                     g.tri_sb)
            for s0, s1 in segs:
                nc.tensor.matmul(ctx[:, s0:s1], g.vs[jk], pr[:, s0:s1],
                                 start=(jk == 0), stop=(jk == nks - 1))
            if jk % 2 == 1:
                grp = next(filler, None)
                if grp is not None:
                    grp()
        dsb = g.rdp.tile([1, AQ], F32, tag="dsb", name="dsb")
        nc.vector.tensor_copy(out=dsb, in_=ctx[64:65, :])
        rd = g.rdp.tile([1, AQ], F32, tag="rd", name="rd")
        nc.vector.reciprocal_approx_fast(rd, dsb)
        rdb = g.rdp.tile([64, AQ], F32, tag="rdb", name="rdb")
        nc.gpsimd.partition_broadcast(rdb, rd)
        tm = g.tmp_.tile([64, AQ], F32, tag="tm", name="tm")
        nc.vector.tensor_mul(tm, ctx[0:64, :],
                             g.gu[p_][Q][64 * hh:64 * hh + 64, :])
        nc.vector.tensor_mul(g.ctxg[p_][Q][64 * hh:64 * hh + 64, :],
                             tm, rdb)
    for grp in filler:
        grp()


def _outproj_groups(nc, g, out, Q):
    """Yield one closure per (s-tile, d-half) output-projection group."""
    for t in range(8):
        for half in range(2):
            def grp(t=t, half=half):
                jc = t * P
                pos = [g.pjp.tile([P, SQ], F32, tag="pj", name="pj")
                       for _ in range(2)]
                for e in range(2):
                    for ni in range(2):
                        n = 2 * half + ni
                        nc.tensor.matmul(
                            pos[ni], g.ctxg[e][Q][:, jc:jc + P],
                            g.wos[e][:, n * SQ:(n + 1) * SQ],
                            start=(e == 0), stop=(e == 1))
                for ni in range(2):
                    n = 2 * half + ni
                    oc = g.ocp.tile([P, SQ], BF16, tag="oc", name="oc")
                    nc.vector.tensor_copy(out=oc, in_=pos[ni])
                    eng = nc.sync if n % 2 == 0 else nc.scalar
                    eng.dma_start(
                        out=out[(8 * Q + t) * P:(8 * Q + t + 1) * P,
                                n * SQ:(n + 1) * SQ],
                        in_=oc)
            yield grp


def build_nc():
    nc = bacc.Bacc("TRN2", target_bir_lowering=False, debug=False,
                   num_devices=NCORE)

    aps = {}
    for nm, shp in (("xt", [D, S]), ("wt", [D, ET]), ("cosq", [P, S]),
                    ("sinq", [P, S]), ("coskv", [P, S]), ("sinkv", [P, S]),
                    ("tri", [P, P]), ("wo", [EL, D]), ("sel", [66, P])):
        aps[nm] = nc.dram_tensor(nm, shp, BF16, kind="ExternalInput").ap()
    out = nc.dram_tensor("out", [S, D], BF16, kind="ExternalOutput").ap()

    g = Ctx()
    with tile.TileContext(nc) as tc, ExitStack() as es:
        g.pp = es.enter_context(tc.tile_pool(name="persist", bufs=1))
        g.sqp = es.enter_context(tc.tile_pool(name="sqp", bufs=2))
        g.nnp = es.enter_context(tc.tile_pool(name="nn", bufs=1))
        g.ropep = es.enter_context(tc.tile_pool(name="rope", bufs=1))
        g.gtp = es.enter_context(tc.tile_pool(name="gt", bufs=2))
        g.prp = es.enter_context(tc.tile_pool(name="pr", bufs=3))
        g.rdp = es.enter_context(tc.tile_pool(name="rd", bufs=1))
        g.tmp_ = es.enter_context(tc.tile_pool(name="tm", bufs=1))
        g.ocp = es.enter_context(tc.tile_pool(name="oc", bufs=3))
        g.pjp = es.enter_context(tc.tile_pool(name="pj", bufs=2, space="PSUM"))
        g.scp = es.enter_context(tc.tile_pool(name="sc", bufs=2, space="PSUM"))
        g.cxp = es.enter_context(tc.tile_pool(name="cx", bufs=1, space="PSUM"))

        _load_persistent(nc, g, aps)

        def cluster(qs):
            for q in qs:
                nc.scalar.activation(g.statsb[q], g.statsb[q], AF.Ln,
                                     bias=g.epsb[0:65, :], scale=1.0 / HD)
            for q in qs:
                nc.scalar.activation(g.R[q], g.statsb[q], AF.Exp, scale=-0.5)

        _proj_q(nc, g, 0)
        _proj_q(nc, g, 1)
        cluster((0, 1))
        rope0 = _rope_pieces(nc, g, 0)
        _proj_q(nc, g, 2, filler=rope0)
        _proj_q(nc, g, 3, filler=rope0)
        for grp in rope0:
            grp()
        cluster((2, 3))
        _attn_Q(nc, g, 0, filler=_rope_pieces(nc, g, 1))
        _attn_Q(nc, g, 1, filler=_outproj_groups(nc, g, out, 0))
        for grp in _outproj_groups(nc, g, out, 1):
            grp()

    nc.compile()
    return nc


def prep_inputs(x, cos, sin, Wq, Wk, Wv, Wo, q_norm_w, k_norm_w):
    """Host-side shard + layout prep. Returns per-core input maps."""
    xtn = np.ascontiguousarray(x.reshape(S, D).T).astype(NBF)

    half = HD // 2
    wq1 = (1.0 + q_norm_w).astype(np.float32)
    wk1 = (1.0 + k_norm_w).astype(np.float32)

    def rotw(w):
        return np.concatenate([w[half:], w[:half]])

    sin_m = sin.copy()
    sin_m[:, :half] = -sin_m[:, :half]
    cos_qh = (cos * wq1).T                     # [64, S]
    sin_qh = (sin_m * rotw(wq1)).T
    cos_kh = (cos * wk1).T
    sin_kh = (sin_m * rotw(wk1)).T
    sin_qh = np.roll(sin_qh, 32, axis=0)
    sin_kh = np.roll(sin_kh, 32, axis=0)
    cosq_t = np.ascontiguousarray(np.tile(cos_qh, (2, 1))).astype(NBF)
    sinq_t = np.ascontiguousarray(np.tile(sin_qh, (2, 1))).astype(NBF)
    coskv_t = np.concatenate([cos_kh, np.ones((HD, S), np.float32)], axis=0)
    sinkv_t = np.concatenate([sin_kh, np.zeros((HD, S), np.float32)], axis=0)
    coskv_t = np.ascontiguousarray(coskv_t).astype(NBF)
    sinkv_t = np.ascontiguousarray(sinkv_t).astype(NBF)

    tri = np.triu(np.ones((P, P), dtype=np.float32)).astype(NBF)

    sel = np.zeros((66, P), dtype=np.float32)
    for b in (0, 32, 64):
        sel[b, 0:64] = 1.0
    for b in (1, 33):
        sel[b, 64:P] = 1.0
    sel = sel.astype(NBF)

    Wqh = Wq.reshape(H, 2 * HD, D)
    in_maps = []
    for c in range(NCORE):
        hs = slice(NHL * c, NHL * (c + 1))
        wq_c = Wqh[hs, :HD, :].reshape(EL, D)
        wg_c = Wqh[hs, HD:, :].reshape(EL, D)
        wk_c = Wk[HD * c:HD * (c + 1), :]
        wv_c = Wv[HD * c:HD * (c + 1), :]
        w_local = np.concatenate([wq_c, wk_c, wv_c, wg_c], axis=0)  # [640, D]
        in_maps.append({
            "xt": xtn,
            "wt": np.ascontiguousarray(w_local.T).astype(NBF),
            "cosq": cosq_t, "sinq": sinq_t,
            "coskv": coskv_t, "sinkv": sinkv_t,
            "tri": tri, "sel": sel,
            "wo": np.ascontiguousarray(Wo[:, EL * c:EL * (c + 1)].T).astype(NBF),
        })
    return in_maps


_NC_CACHE = {}


def get_nc():
    if "nc" not in _NC_CACHE:
        _NC_CACHE["nc"] = build_nc()
    return _NC_CACHE["nc"]


def run(in_maps, trace=False, **kw):
    nc = get_nc()
    return run_bass_kernel_spmd(nc, in_maps, list(range(NCORE)),
                                trace=trace, **kw)


def kernel(x, mask, cos, sin, Wq, Wk, Wv, Wo, q_norm_w, k_norm_w):
    in_maps = prep_inputs(np.asarray(x, dtype=np.float32), np.asarray(cos),
                          np.asarray(sin), np.asarray(Wq), np.asarray(Wk),
                          np.asarray(Wv), np.asarray(Wo),
                          np.asarray(q_norm_w), np.asarray(k_norm_w))
    res = run(in_maps)
    acc = np.zeros((S, D), dtype=np.float32)
    for r in res.results:
        acc += np.asarray(r["out"], dtype=np.float32)
    return acc.reshape(1, S, D)
